# revision 32
# baseline (speedup 1.0000x reference)
# kernel.py — fused causal ReLU-attention (qkv proj + q@k^T + relu/causal + @v)
# for Trainium2, 8 NeuronCores, batch-parallel (1 batch element per core).
#
# v2: host-side pre-transpose of x (feeds xT directly) and host-side
# de-transpose of the output (kernel emits yT), removing all 96 PE
# transposes + their evacuation copies; weights host-repacked into
# first-use order so QKV matmuls start ~2.5us into the kernel.
#
# Self-contained: hardcodes shapes B,T,C = 8,1024,768, nh=12, hs=64.
import os
import sys

for p in ("/opt/trn_rl_repo", "/root/.axon_site", "/root/.axon_site/_ro/trn_rl_repo"):
    if os.path.isdir(p) and p not in sys.path:
        sys.path.append(p)

import numpy as np

import concourse.bass as bass
import concourse.mybir as mybir
import concourse.tile as tile
from concourse import bacc
from concourse import bass_utils

F32 = mybir.dt.float32
BF16 = mybir.dt.bfloat16
AF = mybir.ActivationFunctionType
ALU = mybir.AluOpType

B, T, C = 8, 1024, 768
NH, HS = 12, 64
SCALE = 1.0 / 8.0  # 1/sqrt(64)
P = 128
NT = T // P    # 8 t-tiles
KC = C // P    # 6 c-tiles (contraction)
NPAIR = NH // 2  # 6 head pairs
TCH = 512      # t1 chunk size
NCH = T // TCH  # 2 chunks

# column-chunk order in which qkT groups are consumed (pairs (j, 6+j))
M_ORDER = [0, 6, 1, 7, 2, 8, 3, 9, 4, 10, 5, 11]


def build_nc(n_cores=8):
    nc = bacc.Bacc("TRN2", target_bir_lowering=False, debug=False,
                   num_devices=n_cores)

    # xT: x pre-transposed on host -> [C, T]
    xt_d = nc.dram_tensor("xt", [C, T], BF16, kind="ExternalInput").ap()
    # wqk: host-repacked [12, 128, 6*128]; wqk[jj*2+s] is column chunk
    # m=M_ORDER[2*jj+s] of W[:, :2C], laid out [p, (k col)]
    wqk_d = nc.dram_tensor("wqk", [12, P, C], BF16, kind="ExternalInput").ap()
    # wv: [6, 128, 768] = W[:, 2C:3C] split by contraction tile, [k, p, col]
    wv_d = nc.dram_tensor("wv", [KC, P, C], BF16, kind="ExternalInput").ap()
    # biases host-prepared: bqk [128, 12] (col m = bias for qkT[m] rows),
    # bv [1, 768] — both contiguous for fast DMA
    bqk_d = nc.dram_tensor("bqk", [P, 12], F32, kind="ExternalInput").ap()
    bv_d = nc.dram_tensor("bv", [1, C], F32, kind="ExternalInput").ap()
    # output: yT [C, T]; host transposes back
    y_d = nc.dram_tensor("y", [C, T], BF16, kind="ExternalOutput").ap()

    with tile.TileContext(nc) as tc:
        _emit(nc, tc, xt_d, wqk_d, wv_d, bqk_d, bv_d, y_d)

    nc.compile()
    return nc


def _emit(nc, tc, xt_d, wqk_d, wv_d, bqk_d, bv_d, y_d):
    from contextlib import ExitStack

    with ExitStack() as ctx:
        pp = ctx.enter_context(tc.tile_pool(name="persist", bufs=1))
        ap_pool = ctx.enter_context(tc.tile_pool(name="attp", bufs=42))
        yt_pool = ctx.enter_context(tc.tile_pool(name="yT", bufs=4))
        qps = ctx.enter_context(
            tc.tile_pool(name="qkv_psum", bufs=2, space="PSUM"))
        a_ps = ctx.enter_context(
            tc.tile_pool(name="att_psum", bufs=6, space="PSUM"))

        # ---- persistent tensors ----
        xT = pp.tile([P, KC * T], BF16, tag="xT", name="xT")       # (k, t)
        wqk = [pp.tile([P, 2 * C], BF16, tag=f"wqk{j}", name=f"wqk{j}")
               for j in range(6)]                                  # (s, k, col)
        wv = pp.tile([P, KC * C], BF16, tag="wv", name="wv")       # (k, col)
        qkT = [pp.tile([P, T], BF16, tag=f"qkT{m}", name=f"qkT{m}")
               for m in range(12)]
        v_sb = [pp.tile([P, C], BF16, tag=f"v{i}", name=f"v{i}")
                for i in range(NT)]

        # ---- input DMAs, ordered by first use, on the two HW queues ----
        def dma_wqk(jj, eng):
            eng.dma_start(
                wqk[jj][:].rearrange("p (s c) -> p s c", s=2),
                wqk_d[2 * jj:2 * jj + 2].rearrange("s p c -> p s c"))

        def dma_xt(h, eng):
            eng.dma_start(
                xT[:].rearrange("p (k t) -> p k t", k=KC)[:, :, TCH * h:TCH * (h + 1)],
                xt_d.rearrange("(k p) t -> p k t", p=P)[:, :, TCH * h:TCH * (h + 1)])

        # sync queue: x first, later wqk pairs after
        dma_xt(0, nc.sync)
        dma_xt(1, nc.sync)
        dma_wqk(2, nc.sync)
        dma_wqk(3, nc.sync)
        dma_wqk(4, nc.sync)
        dma_wqk(5, nc.sync)
        # scalar queue: first wqk pairs, biases, v weights
        dma_wqk(0, nc.scalar)
        dma_wqk(1, nc.scalar)
        bqk = pp.tile([P, 12], F32, tag="bqk", name="bqk")
        nc.scalar.dma_start(bqk[:], bqk_d)
        bv_row = pp.tile([1, C], F32, tag="bvrow", name="bvrow")
        nc.scalar.dma_start(bv_row[:], bv_d)
        nc.scalar.dma_start(
            wv[:, 0:3 * C].rearrange("p (k c) -> p k c", k=3),
            wv_d[0:3].rearrange("k p c -> p k c"))
        nc.scalar.dma_start(
            wv[:, 3 * C:].rearrange("p (k c) -> p k c", k=3),
            wv_d[3:6].rearrange("k p c -> p k c"))

        # ---- PE warmup: ramp the clock while input DMAs stream ----
        scratch = pp.tile([P, TCH], BF16, tag="warm", name="warm")
        nc.gpsimd.memset(scratch[:], 0.0)
        for _ in range(16):
            wps = qps.tile([P, TCH], F32, tag="qkvps", name="qkvps")
            nc.tensor.matmul(wps[:], scratch[:, 0:P], scratch[:],
                             start=True, stop=True)

        # ---- constants (gpsimd) ----
        bv = pp.tile([P, C], F32, tag="bv", name="bv")
        nc.gpsimd.partition_broadcast(bv[:], bv_row[0:1, :])

        # master relu/causal mask, pre-scaled by SCALE:
        #   cols [0,384) = 0 ; [384,512) = (col-384>=part ? SCALE : 0) ;
        #   [512,896) = SCALE
        mstr = pp.tile([P, 896], F32, tag="mstr", name="mstr")
        nc.gpsimd.memset(mstr[:, 0:384], 0.0)
        nc.gpsimd.memset(mstr[:, 384:896], SCALE)
        nc.gpsimd.affine_select(
            out=mstr[:, 384:512], in_=mstr[:, 384:512],
            compare_op=ALU.is_ge, fill=0.0, base=0,
            pattern=[[1, P]], channel_multiplier=-1)

        # 0/1 fix mask: cols [0,128)=0 ; [128,256) = (col-128>=part ? 1 : 0)
        mstr2 = pp.tile([P, 256], BF16, tag="mstr2", name="mstr2")
        nc.gpsimd.memset(mstr2[:, 0:128], 0.0)
        nc.gpsimd.memset(mstr2[:, 128:256], 1.0)
        nc.gpsimd.affine_select(
            out=mstr2[:, 128:256], in_=mstr2[:, 128:256],
            compare_op=ALU.is_ge, fill=0.0, base=0,
            pattern=[[1, P]], channel_multiplier=-1)

        def wqk_slice(m, k):
            jj = M_ORDER.index(m) // 2
            s = M_ORDER.index(m) % 2
            return wqk[jj][:, C * s + P * k: C * s + P * (k + 1)]

        def emit_qk_group(m, t):
            ps = qps.tile([P, TCH], F32, tag="qkvps", name="qkvps")
            for k in range(KC):
                nc.tensor.matmul(
                    ps[:],
                    wqk_slice(m, k),
                    xT[:, T * k + TCH * t: T * k + TCH * (t + 1)],
                    start=(k == 0), stop=(k == KC - 1))
            nc.scalar.activation(qkT[m][:, TCH * t:TCH * (t + 1)],
                                 ps[:], AF.Identity,
                                 bias=bqk[:, m:m + 1])

        # ---- pair 0's and pair 1's-t0 qk groups, then v projection ----
        emit_qk_group(0, 0)
        emit_qk_group(6, 0)
        emit_qk_group(1, 0)
        emit_qk_group(7, 0)
        emit_qk_group(0, 1)
        emit_qk_group(6, 1)

        def att_piece_params(c):
            c_lo = TCH * c
            out = []
            for r in range((TCH * (c + 1)) // P):
                t2_0 = P * r
                off = max(0, t2_0 - c_lo)
                offp = min(off, TCH - 128)   # widen tails to N>=128
                z = off - offp
                n = TCH - offp
                out.append((r, offp, z, n, t2_0 >= c_lo))
            return out

        evac_rr = [0]

        def evac_piece(ps, at, z, n, diag):
            """relu+scale+causal-mask evacuation, alternating ACT/DVE."""
            k = evac_rr[0] % 2
            evac_rr[0] += 1
            if diag and k == 0:
                nc.vector.scalar_tensor_tensor(
                    at[:, 0:n], ps[:, 0:n], 0.0,
                    mstr[:, 384 - z:384 - z + n],
                    ALU.max, ALU.mult)
            elif diag:
                # ACT relu, then a short DVE fix zeroes the causal wedge
                nc.scalar.activation(at[:, 0:n], ps[:, 0:n],
                                     AF.Relu, scale=SCALE)
                nc.vector.tensor_tensor(
                    at[:, 0:z + P], at[:, 0:z + P],
                    mstr2[:, P - z:2 * P - z + z], ALU.mult)
            elif k == 0:
                nc.scalar.activation(at[:, 0:n], ps[:, 0:n],
                                     AF.Relu, scale=SCALE)
            else:
                nc.vector.tensor_scalar(
                    at[:, 0:n], ps[:, 0:n], SCALE, 0.0,
                    ALU.mult, ALU.max)

        def emit_qk_pieces(j, c, qt, kt):
            """qk pair matmul block for one (pair, chunk); for c=1 split
            8+8 with the next pair's t1 chains between."""
            c_lo = TCH * c
            c_hi = TCH * (c + 1)
            chunk_pieces = []
            for pi, (r, offp, z, n, diag) in enumerate(att_piece_params(c)):
                if (c == 1 and pi == 4) and j + 1 < NPAIR:
                    emit_qk_group(j + 1, 1)
                    emit_qk_group(7 + j, 1)
                for hh in range(2):
                    h0 = 64 * hh
                    ps = a_ps.tile([P, TCH], F32, tag="aps", name="aps")
                    nc.tensor.matmul(
                        ps[:, 0:n],
                        kt[h0:h0 + 64, P * r:P * r + P],
                        qt[h0:h0 + 64, c_lo + offp:c_hi],
                        start=True, stop=True,
                        tile_position=(h0, 0))
                    at = ap_pool.tile([P, TCH], BF16, tag="attp",
                                      name="attp")
                    evac_piece(ps, at, z, n, diag)
                    chunk_pieces.append((r, hh, offp, n, at))
            return chunk_pieces

        stash = {}
        for i in range(NT):
            for (n0, n1) in ((0, 512), (512, 768)):
                ps = qps.tile([P, TCH], F32, tag="qkvps", name="qkvps")
                for k in range(KC):
                    nc.tensor.matmul(
                        ps[:, 0:n1 - n0],
                        xT[:, T * k + P * i: T * k + P * (i + 1)],
                        wv[:, C * k + n0: C * k + n1],
                        start=(k == 0), stop=(k == KC - 1))
                nc.vector.tensor_tensor(
                    v_sb[i][:, n0:n1], ps[:, 0:n1 - n0],
                    bv[:, n0:n1], ALU.add)
            if i == 3:
                # pair 0's c0 qk block rides the v phase: ACT/DVE are
                # mostly idle here so its evacuations are free
                stash[(0, 0)] = emit_qk_pieces(0, 0, qkT[0], qkT[6])

        # ======= attention: per pair; pair j+1's qkT chains batched inside
        # pair j (pair->full-matmul mode transitions cost ~200ns each, so
        # keep same-mode matmuls contiguous) =======
        for j in range(NPAIR):
            qt, kt = qkT[j], qkT[6 + j]
            yT2 = yt_pool.tile([P, T], BF16, tag="yT", name="yT")

            if j == 4:
                # pair 5 has no chain cover of its own; its c0 qk block
                # rides pair 4's window instead (inputs ready by then)
                stash[(5, 0)] = emit_qk_pieces(5, 0, qkT[5], qkT[11])

            # last pair: process the long chunk first so the kernel ends
            # on the short one (shorter drain tail)
            for c in ((1, 0) if j == NPAIR - 1 else (0, 1)):
                c_lo = TCH * c
                c_hi = TCH * (c + 1)
                rmax = c_hi // P

                chunk_pieces = stash.pop((j, c), None)
                if chunk_pieces is None:
                    chunk_pieces = emit_qk_pieces(j, c, qt, kt)

                # --- later pairs' qkT chains (full-array mode), batched ---
                if c == 0 and j + 2 < NPAIR:
                    emit_qk_group(j + 2, 0)
                    emit_qk_group(8 + j, 0)

                # --- av for this chunk; col-packed pair per piece ---
                yp = [a_ps.tile([P, TCH], F32, tag="aps",
                                name="aps") for _ in range(2)]
                for (r2, hh, offp2, n2, at2) in chunk_pieces:
                    h0 = 64 * hh
                    nc.tensor.matmul(
                        yp[hh][h0:h0 + 64, offp2:offp2 + n2],
                        v_sb[r2][:, P * j + h0:P * j + h0 + 64],
                        at2[:, 0:n2],
                        start=(r2 == 0), stop=(r2 == rmax - 1),
                        tile_position=(0, h0))
                for hh in range(2):
                    h0 = 64 * hh
                    if (c + hh) % 2 == 0:
                        nc.scalar.activation(
                            yT2[h0:h0 + 64, c_lo:c_hi],
                            yp[hh][h0:h0 + 64, :], AF.Copy)
                    else:
                        nc.vector.tensor_copy(
                            yT2[h0:h0 + 64, c_lo:c_hi],
                            yp[hh][h0:h0 + 64, :])
                # yT2 chunk -> y rows [128j : 128(j+1)], cols c_lo:c_hi
                nc.sync.dma_start(y_d[P * j:P * (j + 1), c_lo:c_hi],
                                  yT2[:, c_lo:c_hi])





def _ensure_ntff_hook():
    """Register the axon NTFF profiling hook if the image's antenv lacks
    axon_hooks (bass_utils hard-imports it on the trace=True path)."""
    import types
    try:
        from antenv import axon_hooks  # noqa: F401
        return
    except ImportError:
        pass
    import antenv
    mod = types.ModuleType("antenv.axon_hooks")
    mod._hook = None

    def set_axon_ntff_profile_hook(h):
        mod._hook = h

    def get_axon_ntff_profile_hook():
        return mod._hook

    mod.set_axon_ntff_profile_hook = set_axon_ntff_profile_hook
    mod.get_axon_ntff_profile_hook = get_axon_ntff_profile_hook
    sys.modules["antenv.axon_hooks"] = mod
    antenv.axon_hooks = mod
    try:
        from trn_agent_boot.trn_boot import _ntff_profile_via_ctypes
        hook = _ntff_profile_via_ctypes("/opt/axon/libaxon_pjrt.so")
        if hook is not None:
            mod._hook = hook
    except Exception:
        pass


_NC_CACHE = None


def _get_nc():
    global _NC_CACHE
    if _NC_CACHE is None:
        _NC_CACHE = build_nc()
    return _NC_CACHE


def kernel(x, W_attn, b_attn, _trace=False):
    import ml_dtypes
    bf16 = ml_dtypes.bfloat16
    x = np.asarray(x, dtype=np.float32)
    w = np.asarray(W_attn).astype(bf16)
    b = np.ascontiguousarray(np.asarray(b_attn, dtype=np.float32))
    assert x.shape == (B, T, C) and w.shape == (C, 3 * C) and b.shape == (3 * C,)

    # host-side repack: xT per batch, W column chunks in first-use order
    xt = np.ascontiguousarray(x.transpose(0, 2, 1).astype(bf16))  # [B, C, T]
    wqk = np.ascontiguousarray(np.stack([
        w[:, P * m:P * (m + 1)].reshape(KC, P, P).transpose(1, 0, 2)
        .reshape(P, C)
        for m in M_ORDER]))                                       # [12, P, C]
    wv = np.ascontiguousarray(w[:, 2 * C:3 * C].reshape(KC, P, C))

    if _trace:
        _ensure_ntff_hook()
    nc = _get_nc()
    bqk_h = np.ascontiguousarray(b[0:2 * C].reshape(12, P).T)   # [128, 12]
    bv_h = np.ascontiguousarray(b[2 * C:3 * C].reshape(1, C))
    in_maps = [{"xt": xt[i], "wqk": wqk, "wv": wv,
                "bqk": bqk_h, "bv": bv_h}
               for i in range(B)]
    res = bass_utils.run_bass_kernel_spmd(
        nc, in_maps, core_ids=list(range(B)), trace=_trace)
    y = np.stack([np.asarray(res.results[i]["y"]).astype(np.float32).T
                  for i in range(B)], axis=0)
    if _trace:
        kernel.last_result = res
    return y


# revision 33
# speedup vs baseline: 1.0151x; 1.0151x over previous
# kernel.py — fused causal ReLU-attention (qkv proj + q@k^T + relu/causal + @v)
# for Trainium2, 8 NeuronCores, batch-parallel (1 batch element per core).
#
# v2: host-side pre-transpose of x (feeds xT directly) and host-side
# de-transpose of the output (kernel emits yT), removing all 96 PE
# transposes + their evacuation copies; weights host-repacked into
# first-use order so QKV matmuls start ~2.5us into the kernel.
#
# Self-contained: hardcodes shapes B,T,C = 8,1024,768, nh=12, hs=64.
import os
import sys

for p in ("/opt/trn_rl_repo", "/root/.axon_site", "/root/.axon_site/_ro/trn_rl_repo"):
    if os.path.isdir(p) and p not in sys.path:
        sys.path.append(p)

import numpy as np

import concourse.bass as bass
import concourse.mybir as mybir
import concourse.tile as tile
from concourse import bacc
from concourse import bass_utils

F32 = mybir.dt.float32
BF16 = mybir.dt.bfloat16
AF = mybir.ActivationFunctionType
ALU = mybir.AluOpType

B, T, C = 8, 1024, 768
NH, HS = 12, 64
SCALE = 1.0 / 8.0  # 1/sqrt(64)
P = 128
NT = T // P    # 8 t-tiles
KC = C // P    # 6 c-tiles (contraction)
NPAIR = NH // 2  # 6 head pairs
TCH = 512      # t1 chunk size
NCH = T // TCH  # 2 chunks

# column-chunk order in which qkT groups are consumed (pairs (j, 6+j))
M_ORDER = [0, 6, 1, 7, 2, 8, 3, 9, 4, 10, 5, 11]


def build_nc(n_cores=8):
    nc = bacc.Bacc("TRN2", target_bir_lowering=False, debug=False,
                   num_devices=n_cores)

    # xT: x pre-transposed on host -> [C, T]
    xt_d = nc.dram_tensor("xt", [C, T], BF16, kind="ExternalInput").ap()
    # wqk: host-repacked [12, 128, 6*128]; wqk[jj*2+s] is column chunk
    # m=M_ORDER[2*jj+s] of W[:, :2C], laid out [p, (k col)]
    wqk_d = nc.dram_tensor("wqk", [12, P, C], BF16, kind="ExternalInput").ap()
    # wv: [6, 128, 768] = W[:, 2C:3C] split by contraction tile, [k, p, col]
    wv_d = nc.dram_tensor("wv", [KC, P, C], BF16, kind="ExternalInput").ap()
    # biases host-prepared: bqk [128, 12] (col m = bias for qkT[m] rows),
    # bv [1, 768] — both contiguous for fast DMA
    bqk_d = nc.dram_tensor("bqk", [P, 12], F32, kind="ExternalInput").ap()
    bv_d = nc.dram_tensor("bv", [1, C], F32, kind="ExternalInput").ap()
    # output: yT [C, T]; host transposes back
    y_d = nc.dram_tensor("y", [C, T], BF16, kind="ExternalOutput").ap()

    with tile.TileContext(nc) as tc:
        _emit(nc, tc, xt_d, wqk_d, wv_d, bqk_d, bv_d, y_d)

    nc.compile()
    return nc


def _emit(nc, tc, xt_d, wqk_d, wv_d, bqk_d, bv_d, y_d):
    from contextlib import ExitStack

    with ExitStack() as ctx:
        pp = ctx.enter_context(tc.tile_pool(name="persist", bufs=1))
        ap_pool = ctx.enter_context(tc.tile_pool(name="attp", bufs=30))
        yt_pool = ctx.enter_context(tc.tile_pool(name="yT", bufs=4))
        qps = ctx.enter_context(
            tc.tile_pool(name="qkv_psum", bufs=2, space="PSUM"))
        a_ps = ctx.enter_context(
            tc.tile_pool(name="att_psum", bufs=6, space="PSUM"))

        # ---- persistent tensors ----
        xT = pp.tile([P, KC * T], BF16, tag="xT", name="xT")       # (k, t)
        wqk = [pp.tile([P, 2 * C], BF16, tag=f"wqk{j}", name=f"wqk{j}")
               for j in range(6)]                                  # (s, k, col)
        wv = pp.tile([P, KC * C], BF16, tag="wv", name="wv")       # (k, col)
        qkT = [pp.tile([P, T], BF16, tag=f"qkT{m}", name=f"qkT{m}")
               for m in range(12)]
        v_sb = [pp.tile([P, C], BF16, tag=f"v{i}", name=f"v{i}")
                for i in range(NT)]

        # ---- input DMAs, ordered by first use, on the two HW queues ----
        def dma_wqk(jj, eng):
            eng.dma_start(
                wqk[jj][:].rearrange("p (s c) -> p s c", s=2),
                wqk_d[2 * jj:2 * jj + 2].rearrange("s p c -> p s c"))

        def dma_xt(h, eng):
            eng.dma_start(
                xT[:].rearrange("p (k t) -> p k t", k=KC)[:, :, TCH * h:TCH * (h + 1)],
                xt_d.rearrange("(k p) t -> p k t", p=P)[:, :, TCH * h:TCH * (h + 1)])

        # sync queue: x first, later wqk pairs after
        dma_xt(0, nc.sync)
        dma_xt(1, nc.sync)
        dma_wqk(2, nc.sync)
        dma_wqk(3, nc.sync)
        dma_wqk(4, nc.sync)
        dma_wqk(5, nc.sync)
        # scalar queue: first wqk pairs, biases, v weights
        dma_wqk(0, nc.scalar)
        dma_wqk(1, nc.scalar)
        bqk = pp.tile([P, 12], F32, tag="bqk", name="bqk")
        nc.scalar.dma_start(bqk[:], bqk_d)
        bv_row = pp.tile([1, C], F32, tag="bvrow", name="bvrow")
        nc.scalar.dma_start(bv_row[:], bv_d)
        nc.scalar.dma_start(
            wv[:, 0:3 * C].rearrange("p (k c) -> p k c", k=3),
            wv_d[0:3].rearrange("k p c -> p k c"))
        nc.scalar.dma_start(
            wv[:, 3 * C:].rearrange("p (k c) -> p k c", k=3),
            wv_d[3:6].rearrange("k p c -> p k c"))

        # ---- PE warmup: ramp the clock while input DMAs stream ----
        scratch = pp.tile([P, TCH], BF16, tag="warm", name="warm")
        nc.gpsimd.memset(scratch[:], 0.0)
        for _ in range(16):
            wps = qps.tile([P, TCH], F32, tag="qkvps", name="qkvps")
            nc.tensor.matmul(wps[:], scratch[:, 0:P], scratch[:],
                             start=True, stop=True)

        # ---- constants (gpsimd) ----
        bv = pp.tile([P, C], F32, tag="bv", name="bv")
        nc.gpsimd.partition_broadcast(bv[:], bv_row[0:1, :])

        # master relu/causal mask, pre-scaled by SCALE:
        #   cols [0,384) = 0 ; [384,512) = (col-384>=part ? SCALE : 0) ;
        #   [512,896) = SCALE
        mstr = pp.tile([P, 896], F32, tag="mstr", name="mstr")
        nc.gpsimd.memset(mstr[:, 0:384], 0.0)
        nc.gpsimd.memset(mstr[:, 384:896], SCALE)
        nc.gpsimd.affine_select(
            out=mstr[:, 384:512], in_=mstr[:, 384:512],
            compare_op=ALU.is_ge, fill=0.0, base=0,
            pattern=[[1, P]], channel_multiplier=-1)

        # 0/1 fix mask: cols [0,128)=0 ; [128,256) = (col-128>=part ? 1 : 0)
        mstr2 = pp.tile([P, 256], BF16, tag="mstr2", name="mstr2")
        nc.gpsimd.memset(mstr2[:, 0:128], 0.0)
        nc.gpsimd.memset(mstr2[:, 128:256], 1.0)
        nc.gpsimd.affine_select(
            out=mstr2[:, 128:256], in_=mstr2[:, 128:256],
            compare_op=ALU.is_ge, fill=0.0, base=0,
            pattern=[[1, P]], channel_multiplier=-1)

        def wqk_slice(m, k):
            jj = M_ORDER.index(m) // 2
            s = M_ORDER.index(m) % 2
            return wqk[jj][:, C * s + P * k: C * s + P * (k + 1)]

        def emit_qk_group(m, t):
            ps = qps.tile([P, TCH], F32, tag="qkvps", name="qkvps")
            for k in range(KC):
                nc.tensor.matmul(
                    ps[:],
                    wqk_slice(m, k),
                    xT[:, T * k + TCH * t: T * k + TCH * (t + 1)],
                    start=(k == 0), stop=(k == KC - 1))
            nc.scalar.activation(qkT[m][:, TCH * t:TCH * (t + 1)],
                                 ps[:], AF.Identity,
                                 bias=bqk[:, m:m + 1])

        # ---- pair 0's and pair 1's-t0 qk groups, then v projection ----
        emit_qk_group(0, 0)
        emit_qk_group(6, 0)
        emit_qk_group(1, 0)
        emit_qk_group(7, 0)
        emit_qk_group(0, 1)
        emit_qk_group(6, 1)

        def att_piece_params(c):
            c_lo = TCH * c
            out = []
            for r in range((TCH * (c + 1)) // P):
                t2_0 = P * r
                off = max(0, t2_0 - c_lo)
                offp = min(off, TCH - 128)   # widen tails to N>=128
                z = off - offp
                n = TCH - offp
                out.append((r, offp, z, n, t2_0 >= c_lo))
            return out

        evac_rr = [0]

        def evac_piece(ps, at, z, n, diag):
            """relu+scale+causal-mask evacuation, alternating ACT/DVE."""
            k = evac_rr[0] % 2
            evac_rr[0] += 1
            if diag and k == 0:
                nc.vector.scalar_tensor_tensor(
                    at[:, 0:n], ps[:, 0:n], 0.0,
                    mstr[:, 384 - z:384 - z + n],
                    ALU.max, ALU.mult)
            elif diag:
                # ACT relu, then a short DVE fix zeroes the causal wedge
                nc.scalar.activation(at[:, 0:n], ps[:, 0:n],
                                     AF.Relu, scale=SCALE)
                nc.vector.tensor_tensor(
                    at[:, 0:z + P], at[:, 0:z + P],
                    mstr2[:, P - z:2 * P - z + z], ALU.mult)
            elif k == 0:
                nc.scalar.activation(at[:, 0:n], ps[:, 0:n],
                                     AF.Relu, scale=SCALE)
            else:
                nc.vector.tensor_scalar(
                    at[:, 0:n], ps[:, 0:n], SCALE, 0.0,
                    ALU.mult, ALU.max)

        def emit_qk_pieces(j, c, qt, kt):
            """qk pair matmul block for one (pair, chunk); for c=1 split
            8+8 with the next pair's t1 chains between."""
            c_lo = TCH * c
            c_hi = TCH * (c + 1)
            chunk_pieces = []
            for pi, (r, offp, z, n, diag) in enumerate(att_piece_params(c)):
                if (c == 1 and pi == 4) and j + 1 < NPAIR:
                    emit_qk_group(j + 1, 1)
                    emit_qk_group(7 + j, 1)
                for hh in range(2):
                    h0 = 64 * hh
                    ps = a_ps.tile([P, TCH], F32, tag="aps", name="aps")
                    nc.tensor.matmul(
                        ps[:, 0:n],
                        kt[h0:h0 + 64, P * r:P * r + P],
                        qt[h0:h0 + 64, c_lo + offp:c_hi],
                        start=True, stop=True,
                        tile_position=(h0, 0))
                    at = ap_pool.tile([P, TCH], BF16, tag="attp",
                                      name="attp")
                    evac_piece(ps, at, z, n, diag)
                    chunk_pieces.append((r, hh, offp, n, at))
            return chunk_pieces

        stash = {}
        for i in range(NT):
            for (n0, n1) in ((0, 512), (512, 768)):
                ps = qps.tile([P, TCH], F32, tag="qkvps", name="qkvps")
                for k in range(KC):
                    nc.tensor.matmul(
                        ps[:, 0:n1 - n0],
                        xT[:, T * k + P * i: T * k + P * (i + 1)],
                        wv[:, C * k + n0: C * k + n1],
                        start=(k == 0), stop=(k == KC - 1))
                nc.vector.tensor_tensor(
                    v_sb[i][:, n0:n1], ps[:, 0:n1 - n0],
                    bv[:, n0:n1], ALU.add)
            if i == 3:
                # pair 0's c0 qk block rides the v phase: ACT/DVE are
                # mostly idle here so its evacuations are free
                stash[(0, 0)] = emit_qk_pieces(0, 0, qkT[0], qkT[6])

        # ======= attention: per pair; pair j+1's qkT chains batched inside
        # pair j (pair->full-matmul mode transitions cost ~200ns each, so
        # keep same-mode matmuls contiguous) =======
        for j in range(NPAIR):
            qt, kt = qkT[j], qkT[6 + j]
            yT2 = yt_pool.tile([P, T], BF16, tag="yT", name="yT")

            # last pair: process the long chunk first so the kernel ends
            # on the short one (shorter drain tail)
            for c in ((1, 0) if j == NPAIR - 1 else (0, 1)):
                c_lo = TCH * c
                c_hi = TCH * (c + 1)
                rmax = c_hi // P

                chunk_pieces = stash.pop((j, c), None)
                if chunk_pieces is None:
                    chunk_pieces = emit_qk_pieces(j, c, qt, kt)

                # --- later pairs' qkT chains (full-array mode), batched ---
                if c == 0 and j + 2 < NPAIR:
                    emit_qk_group(j + 2, 0)
                    emit_qk_group(8 + j, 0)

                # --- av for this chunk; col-packed pair per piece ---
                yp = [a_ps.tile([P, TCH], F32, tag="aps",
                                name="aps") for _ in range(2)]
                for (r2, hh, offp2, n2, at2) in chunk_pieces:
                    h0 = 64 * hh
                    nc.tensor.matmul(
                        yp[hh][h0:h0 + 64, offp2:offp2 + n2],
                        v_sb[r2][:, P * j + h0:P * j + h0 + 64],
                        at2[:, 0:n2],
                        start=(r2 == 0), stop=(r2 == rmax - 1),
                        tile_position=(0, h0))
                for hh in range(2):
                    h0 = 64 * hh
                    if (c + hh) % 2 == 0:
                        nc.scalar.activation(
                            yT2[h0:h0 + 64, c_lo:c_hi],
                            yp[hh][h0:h0 + 64, :], AF.Copy)
                    else:
                        nc.vector.tensor_copy(
                            yT2[h0:h0 + 64, c_lo:c_hi],
                            yp[hh][h0:h0 + 64, :])
                # yT2 chunk -> y rows [128j : 128(j+1)], cols c_lo:c_hi
                nc.sync.dma_start(y_d[P * j:P * (j + 1), c_lo:c_hi],
                                  yT2[:, c_lo:c_hi])





def _ensure_ntff_hook():
    """Register the axon NTFF profiling hook if the image's antenv lacks
    axon_hooks (bass_utils hard-imports it on the trace=True path)."""
    import types
    try:
        from antenv import axon_hooks  # noqa: F401
        return
    except ImportError:
        pass
    import antenv
    mod = types.ModuleType("antenv.axon_hooks")
    mod._hook = None

    def set_axon_ntff_profile_hook(h):
        mod._hook = h

    def get_axon_ntff_profile_hook():
        return mod._hook

    mod.set_axon_ntff_profile_hook = set_axon_ntff_profile_hook
    mod.get_axon_ntff_profile_hook = get_axon_ntff_profile_hook
    sys.modules["antenv.axon_hooks"] = mod
    antenv.axon_hooks = mod
    try:
        from trn_agent_boot.trn_boot import _ntff_profile_via_ctypes
        hook = _ntff_profile_via_ctypes("/opt/axon/libaxon_pjrt.so")
        if hook is not None:
            mod._hook = hook
    except Exception:
        pass


_NC_CACHE = None


def _get_nc():
    global _NC_CACHE
    if _NC_CACHE is None:
        _NC_CACHE = build_nc()
    return _NC_CACHE


def kernel(x, W_attn, b_attn, _trace=False):
    import ml_dtypes
    bf16 = ml_dtypes.bfloat16
    x = np.asarray(x, dtype=np.float32)
    w = np.asarray(W_attn).astype(bf16)
    b = np.ascontiguousarray(np.asarray(b_attn, dtype=np.float32))
    assert x.shape == (B, T, C) and w.shape == (C, 3 * C) and b.shape == (3 * C,)

    # host-side repack: xT per batch, W column chunks in first-use order
    xt = np.ascontiguousarray(x.transpose(0, 2, 1).astype(bf16))  # [B, C, T]
    wqk = np.ascontiguousarray(np.stack([
        w[:, P * m:P * (m + 1)].reshape(KC, P, P).transpose(1, 0, 2)
        .reshape(P, C)
        for m in M_ORDER]))                                       # [12, P, C]
    wv = np.ascontiguousarray(w[:, 2 * C:3 * C].reshape(KC, P, C))

    if _trace:
        _ensure_ntff_hook()
    nc = _get_nc()
    bqk_h = np.ascontiguousarray(b[0:2 * C].reshape(12, P).T)   # [128, 12]
    bv_h = np.ascontiguousarray(b[2 * C:3 * C].reshape(1, C))
    in_maps = [{"xt": xt[i], "wqk": wqk, "wv": wv,
                "bqk": bqk_h, "bv": bv_h}
               for i in range(B)]
    res = bass_utils.run_bass_kernel_spmd(
        nc, in_maps, core_ids=list(range(B)), trace=_trace)
    y = np.stack([np.asarray(res.results[i]["y"]).astype(np.float32).T
                  for i in range(B)], axis=0)
    if _trace:
        kernel.last_result = res
    return y
